# revision 1
# baseline (speedup 1.0000x reference)
"""Trainium2 Bass kernel for the bidirectional GRU-ODE (nn_CODEBiGRU).

Strategy (8-way tensor parallel, DVE-matvec formulation):
  - Every matvec is row-sharded: core c computes output rows [512c, 512c+512).
  - Matvecs run on the vector engine as full-width tensor_mul + reduce_sum over
    (128, 4096) tiles (few large instructions), with the rhs vector physically
    replicated across partitions via a broadcast DMA.
  - Both RK4 chains (forward/backward) are carried together; weights are cast
    to bf16 on the host and kept SBUF-resident.
  - After each matvec the 512-row slices are all-gathered (ncfw AllGather);
    RK4 state updates run replicated in fp32 on small tiled buffers.
"""
import sys
import numpy as np

sys.path.insert(0, "/opt/trn_rl_repo")

import ml_dtypes  # noqa: E402
import concourse.bass as bass  # noqa: E402
import concourse.tile as tile  # noqa: E402
from concourse import bacc, mybir, bass_utils  # noqa: E402

NCORES = 8
NH = 4096
R = NH // NCORES       # rows per core (512)
MT = R // 128          # m-tiles per core (4)
KT = NH // 128         # k-tiles of a full vector (32)
NSTEP = 15             # RK4 steps per chain
F32 = mybir.dt.float32
BF16 = mybir.dt.bfloat16
AF = mybir.ActivationFunctionType
ALU = mybir.AluOpType
AX = mybir.AxisListType
GROUP = [list(range(NCORES))]


def _build(niters=1):
    nc = bacc.Bacc("TRN2", target_bir_lowering=False, debug=False,
                   num_devices=NCORES)

    # ---- kernel I/O ----
    w12_d = nc.dram_tensor("w12", [128, 2 * MT * NH], BF16, kind="ExternalInput")
    wg_d = nc.dram_tensor("wg", [128, MT * 2 * NH], BF16, kind="ExternalInput")
    wo_d = nc.dram_tensor("wo", [128, MT * 2 * NH], BF16, kind="ExternalInput")
    x2_d = nc.dram_tensor("x2", [2, NH], BF16, kind="ExternalInput")
    h0_d = nc.dram_tensor("h0", [128, KT, 2], F32, kind="ExternalInput")
    bf1_d = nc.dram_tensor("bf1", [128, MT, 2], F32, kind="ExternalInput")
    bf2_d = nc.dram_tensor("bf2", [128, MT, 2], F32, kind="ExternalInput")
    bg_d = nc.dram_tensor("bg", [128, MT, 2], F32, kind="ExternalInput")
    bo_d = nc.dram_tensor("bo", [128, MT], F32, kind="ExternalInput")
    coef_d = nc.dram_tensor("coef", [128, NSTEP, 3, 2], F32, kind="ExternalInput")

    o_slice = nc.dram_tensor("o_slice", [R], F32, kind="ExternalOutput")
    hf_out = nc.dram_tensor("hf_out", [NH], F32, kind="ExternalOutput")
    hb_out = nc.dram_tensor("hb_out", [NH], F32, kind="ExternalOutput")

    with tile.TileContext(nc) as tc:
        with tc.tile_pool(name="wts", bufs=1) as wts, \
             tc.tile_pool(name="vec", bufs=1) as vec, \
             tc.tile_pool(name="dram", bufs=4, space="DRAM") as dram:

            # ---- persistent SBUF tensors ----
            w12 = wts.tile([128, 2 * MT * NH], BF16, tag="w12")     # 64KB/p
            wg = wts.tile([128, MT, 2 * NH], BF16, tag="wg")        # 64KB/p
            rep = wts.tile([128, 2, 2 * NH], BF16, tag="rep")       # 32KB/p
            scr = wts.tile([128, 2 * NH], F32, tag="scr")           # 32KB/p

            h = vec.tile([128, KT, 2], F32, tag="h")
            hstage = vec.tile([128, KT, 2], BF16, tag="hstage")
            kfull = vec.tile([128, KT, 2], F32, tag="kfull")
            S = vec.tile([128, KT, 2], F32, tag="S")
            tmp = vec.tile([128, KT, 2], F32, tag="tmp")
            u_loc = vec.tile([128, MT, 2], F32, tag="u_loc")
            t_loc = vec.tile([128, MT, 2], BF16, tag="t_loc")
            k_loc = vec.tile([128, MT, 2], F32, tag="k_loc")
            g_loc = vec.tile([128, MT, 2], F32, tag="g_loc")
            gfull = vec.tile([128, KT, 2], F32, tag="gfull")
            hh_loc = vec.tile([128, MT, 2], F32, tag="hh_loc")
            hhfull = vec.tile([128, KT, 2], F32, tag="hhfull")
            ght = vec.tile([128, KT, 2], BF16, tag="ght")
            hn_bf = vec.tile([128, KT, 2], BF16, tag="hn_bf")
            o_loc = vec.tile([128, MT], F32, tag="o_loc")
            bf1 = vec.tile([128, MT, 2], F32, tag="bf1")
            bf2 = vec.tile([128, MT, 2], F32, tag="bf2")
            bg = vec.tile([128, MT, 2], F32, tag="bg")
            bo = vec.tile([128, MT], F32, tag="bo")
            coef = vec.tile([128, NSTEP, 3, 2], F32, tag="coef")

            # weight views
            vw = w12[:].rearrange("p (a m k) -> p a m k", a=2, m=MT)   # ODE W1/W2
            vo = w12[:].rearrange("p (m k) -> p m k", m=MT)            # h2o (post-ODE)

            for _it in range(niters):
                # ---- load inputs ----
                nc.sync.dma_start(w12[:], w12_d[:])
                nc.sync.dma_start(wg[:].rearrange("p m k -> p (m k)"), wg_d[:])
                nc.sync.dma_start(h[:], h0_d[:])
                nc.sync.dma_start(bf1[:], bf1_d[:])
                nc.sync.dma_start(bf2[:], bf2_d[:])
                nc.sync.dma_start(bg[:], bg_d[:])
                nc.sync.dma_start(bo[:], bo_d[:])
                nc.sync.dma_start(coef[:], coef_d[:])

                scr2 = scr[:].rearrange("p (c k) -> p c k", c=2)

                def mv(w_ap_fn, rep_ap, out_loc, bias, width):
                    """out_loc[:,mt,ch] = sum_k w(mt)[:,k]*rep[ch,k] + bias[mt,ch]"""
                    if width == NH:
                        # one mult+reduce per m-tile covering both chains
                        for mt in range(MT):
                            wb = w_ap_fn(mt).rearrange(
                                "p (one k) -> p one k", one=1
                            ).broadcast_to([128, 2, width])
                            nc.vector.tensor_mul(scr2[:], wb, rep_ap[:, :, :width])
                            nc.vector.reduce_sum(out_loc[:, mt, :], scr2[:],
                                                 axis=AX.X)
                    else:
                        for mt in range(MT):
                            for ch in range(2):
                                nc.vector.tensor_mul(scr[:, :width], w_ap_fn(mt),
                                                     rep_ap[:, ch, :width])
                                nc.vector.reduce_sum(out_loc[:, mt, ch:ch + 1],
                                                     scr[:, :width], axis=AX.X)
                    nc.vector.tensor_add(out_loc[:], out_loc[:], bias[:])

                def stage_to_rep(src_bf_tiled, tag):
                    """tiled (128,KT,2) bf16 -> DRAM -> broadcast into rep[:, ch, :NH]"""
                    d = dram.tile([NH, 2], BF16, tag=f"rt_{tag}", name=f"rt_{tag}")
                    nc.sync.dma_start(
                        d[:].rearrange("(kt kp) ch -> kp kt ch", kp=128),
                        src_bf_tiled[:])
                    for ch in range(2):
                        nc.sync.dma_start(
                            rep[:, ch, :NH],
                            d[:, ch].partition_broadcast(128))

                def allgather(loc_ap, dt, tag, full_tiled=None, to_rep=False):
                    inb = dram.tile([R, 2], dt, tag=f"agi_{tag}", name=f"agi_{tag}")
                    outb = dram.tile([NH, 2], dt, tag=f"ago_{tag}", name=f"ago_{tag}")
                    nc.sync.dma_start(
                        inb[:].rearrange("(mt mf) ch -> mf mt ch", mf=128), loc_ap)
                    nc.gpsimd.collective_compute(
                        "AllGather", ALU.bypass, replica_groups=GROUP,
                        ins=[inb.opt()], outs=[outb.opt()])
                    if to_rep:
                        for ch in range(2):
                            nc.sync.dma_start(
                                rep[:, ch, :NH],
                                outb[:, ch].partition_broadcast(128))
                    if full_tiled is not None:
                        nc.sync.dma_start(
                            full_tiled[:],
                            outb[:].rearrange("(kt kp) ch -> kp kt ch", kp=128))

                # initial hstage = cast(h); replicate into rep
                nc.vector.tensor_copy(hstage[:], h[:])
                stage_to_rep(hstage, "hs")

                # ================= ODE phase =================
                for s in range(NSTEP):
                    for q in range(4):
                        # u = W1 @ rep + b1 ; t = tanh(u)
                        mv(lambda mt: vw[:, 0, mt, :], rep, u_loc, bf1, NH)
                        nc.scalar.activation(t_loc[:], u_loc[:], AF.Tanh)
                        allgather(t_loc[:], BF16, "t", to_rep=True)
                        # k = W2 @ rep + b2
                        mv(lambda mt: vw[:, 1, mt, :], rep, k_loc, bf2, NH)
                        allgather(k_loc[:], F32, "k", full_tiled=kfull)

                        # S = k1 + 2 k2 + 2 k3 + k4
                        if q == 0:
                            nc.vector.tensor_copy(S[:], kfull[:])
                        elif q in (1, 2):
                            nc.vector.tensor_scalar_mul(tmp[:], kfull[:], 2.0)
                            nc.vector.tensor_add(S[:], S[:], tmp[:])
                        else:
                            nc.vector.tensor_add(S[:], S[:], kfull[:])

                        def cbr(cidx):
                            return coef[:, s, cidx, :].rearrange(
                                "p (one ch) -> p one ch", one=1
                            ).broadcast_to([128, KT, 2])

                        if q < 3:
                            nc.vector.tensor_mul(tmp[:], kfull[:], cbr(0 if q < 2 else 1))
                            nc.vector.tensor_add(hstage[:], h[:], tmp[:])
                        else:
                            nc.vector.tensor_mul(tmp[:], S[:], cbr(2))
                            nc.vector.tensor_add(h[:], h[:], tmp[:])
                            nc.vector.tensor_copy(hstage[:], h[:])
                        if not (s == NSTEP - 1 and q == 3):
                            # final staged vector is consumed by the GRU phase
                            # (via rt_h2), never by another MM1 read of rep
                            stage_to_rep(hstage, "hs")

                # ================= GRU phase =================
                # rep is dead now; reuse its slot as the (128, 2, 2NH) GRU rhs:
                # [x | h] per chain. x part loaded once (broadcast), h part per call.
                for ch in range(2):
                    nc.sync.dma_start(rep[:, ch, :NH],
                                      x2_d[ch, :].partition_broadcast(128))
                hd = dram.tile([NH, 2], BF16, tag="rt_h2", name="rt_h2")
                nc.sync.dma_start(
                    hd[:].rearrange("(kt kp) ch -> kp kt ch", kp=128), hstage[:])
                for ch in range(2):
                    nc.sync.dma_start(
                        rep[:, ch, NH:],
                        hd[:, ch].partition_broadcast(128))

                # g = sigmoid(i2h @ [x, h] + i2h_b)
                mv(lambda mt: wg[:, mt, :], rep, u_loc, bg, 2 * NH)
                nc.scalar.activation(g_loc[:], u_loc[:], AF.Sigmoid)
                allgather(g_loc[:], F32, "g", full_tiled=gfull)
                # gh = g * h (tiled, bf16) -> rep h-part
                nc.vector.tensor_mul(ght[:], gfull[:], h[:])
                ghd = dram.tile([NH, 2], BF16, tag="rt_gh", name="rt_gh")
                nc.sync.dma_start(
                    ghd[:].rearrange("(kt kp) ch -> kp kt ch", kp=128), ght[:])
                for ch in range(2):
                    nc.sync.dma_start(
                        rep[:, ch, NH:],
                        ghd[:, ch].partition_broadcast(128))
                # h_hat = tanh(i2h @ [x, g*h] + i2h_b)
                mv(lambda mt: wg[:, mt, :], rep, u_loc, bg, 2 * NH)
                nc.scalar.activation(hh_loc[:], u_loc[:], AF.Tanh)
                allgather(hh_loc[:], F32, "hh", full_tiled=hhfull)
                # h_new = hh + g*(h - hh)
                nc.vector.tensor_sub(tmp[:], h[:], hhfull[:])
                nc.vector.tensor_mul(tmp[:], gfull[:], tmp[:])
                nc.vector.tensor_add(h[:], hhfull[:], tmp[:])
                nc.vector.tensor_copy(hn_bf[:], h[:])

                nc.sync.dma_start(hf_out[:].rearrange("(kt kp) -> kp kt", kp=128),
                                  h[:, :, 0])
                nc.sync.dma_start(hb_out[:].rearrange("(kt kp) -> kp kt", kp=128),
                                  h[:, :, 1])

                # ================= output projection =================
                # overwrite w12 with h2o weights; build rhs [h_f ; h_b] in rep[:,0,:]
                nc.sync.dma_start(w12[:], wo_d[:])
                hnd = dram.tile([NH, 2], BF16, tag="rt_hn", name="rt_hn")
                nc.sync.dma_start(
                    hnd[:].rearrange("(kt kp) ch -> kp kt ch", kp=128), hn_bf[:])
                for ch in range(2):
                    nc.sync.dma_start(
                        rep[:, 0, ch * NH:(ch + 1) * NH],
                        hnd[:, ch].partition_broadcast(128))
                for mt in range(MT):
                    nc.vector.tensor_mul(scr[:], vo[:, mt, :], rep[:, 0, :])
                    nc.vector.reduce_sum(o_loc[:, mt:mt + 1], scr[:], axis=AX.X)
                nc.vector.tensor_add(o_loc[:], o_loc[:], bo[:])
                nc.sync.dma_start(o_slice[:].rearrange("(mt mf) -> mf mt", mf=128),
                                  o_loc[:])

    nc.compile()
    return nc


_CACHE = {}


def _get_nc(niters=1):
    key = f"nc{niters}"
    if key not in _CACHE:
        _CACHE[key] = _build(niters)
    return _CACHE[key]


def _rows_bf16(W, c):
    """W (out, in) fp32 -> (128, MT, in) bf16 row-shard for core c: [mf, mt, k]."""
    sl = W[c * R:(c + 1) * R, :].astype(ml_dtypes.bfloat16)
    r = sl.reshape(MT, 128, W.shape[1])
    return np.ascontiguousarray(r.transpose(1, 0, 2))


def _bvec2(vec, c):
    """bias slice for core c -> (128, MT, 2) fp32 (replicated over chains)."""
    b = vec[c * R:(c + 1) * R].reshape(MT, 128).T.astype(np.float32)
    return np.ascontiguousarray(np.repeat(b[:, :, None], 2, axis=2))


def kernel(x_f, x_b, h_f, h_b, t_f, t_b,
           i2h_W, i2h_b, h2o_W, h2o_b, f_W1, f_b1, f_W2, f_b2):
    x_f = np.asarray(x_f, np.float32); x_b = np.asarray(x_b, np.float32)
    h_f = np.asarray(h_f, np.float32); h_b = np.asarray(h_b, np.float32)
    t_f = np.asarray(t_f, np.float32); t_b = np.asarray(t_b, np.float32)
    i2h_W = np.asarray(i2h_W, np.float32); i2h_b = np.asarray(i2h_b, np.float32)
    h2o_W = np.asarray(h2o_W, np.float32); h2o_b = np.asarray(h2o_b, np.float32)
    f_W1 = np.asarray(f_W1, np.float32); f_b1 = np.asarray(f_b1, np.float32)
    f_W2 = np.asarray(f_W2, np.float32); f_b2 = np.asarray(f_b2, np.float32)

    nc = _get_nc(int(_CACHE.get('niters', 1)))

    x2 = np.stack([x_f.reshape(-1), x_b.reshape(-1)]).astype(ml_dtypes.bfloat16)
    h0 = np.stack([h_f.reshape(KT, 128).T, h_b.reshape(KT, 128).T],
                  axis=-1).astype(np.float32)
    coef = np.zeros((NSTEP, 3, 2), np.float32)
    for ch, t in enumerate([t_f, t_b]):
        dt = (t[1:] - t[:-1]).astype(np.float32)
        coef[:, 0, ch] = (dt * np.float32(0.5)).astype(np.float32)
        coef[:, 1, ch] = dt
        coef[:, 2, ch] = (dt / np.float32(6.0)).astype(np.float32)
    coef_b = np.ascontiguousarray(
        np.broadcast_to(coef[None], (128, NSTEP, 3, 2)), dtype=np.float32)

    in_maps = []
    for c in range(NCORES):
        w12 = np.stack([_rows_bf16(f_W1, c), _rows_bf16(f_W2, c)], axis=1)
        in_maps.append({
            "w12": w12.reshape(128, -1),
            "wg": _rows_bf16(i2h_W, c).reshape(128, -1),
            "wo": _rows_bf16(h2o_W, c).reshape(128, -1),
            "x2": x2, "h0": h0,
            "bf1": _bvec2(f_b1, c), "bf2": _bvec2(f_b2, c),
            "bg": _bvec2(i2h_b, c),
            "bo": np.ascontiguousarray(
                h2o_b[c * R:(c + 1) * R].reshape(MT, 128).T, dtype=np.float32),
            "coef": coef_b,
        })

    res = bass_utils.run_bass_kernel_spmd(nc, in_maps, core_ids=list(range(NCORES)))
    _CACHE["last_results"] = res

    out = np.concatenate([res.results[c]["o_slice"] for c in range(NCORES)])
    hf = res.results[0]["hf_out"]
    hb = res.results[0]["hb_out"]
    return out, hf, hb

